# revision 1
# baseline (speedup 1.0000x reference)
"""Bass/Tile GroupedQueryAttention kernel for Trainium2, 8-core head-sharded.

Problem: B=1, S=2048, D=2048, HQ=32 query heads, HKV=8 KV heads, HD=64.
Sharding: core g owns KV head g and its R=4 query heads (reference grouping:
kv head g serves query heads g*R..(g+1)*R-1).  The output projection is
row-sharded (each core multiplies its heads' attention output by the matching
256-row slice of Wo) and the host sums the 8 partial [S, D] outputs.

Everything on-chip runs with the "transposed" operand layouts so that no
on-chip transposes of activations are needed:
  - host supplies xT = x.T (bf16) so the d-contraction is on partitions
  - QT[c, s], KT[c, k], VT[vd, k] come straight out of the projections
    (V is then PE-transposed into natural [k, vd] layout in 128-chunks)
  - scores are computed transposed: ST[k, q] = KT.T @ QT with two heads
    row-packed on the PE (K=64 each, array rows 0-63 / 64-127)
  - exp(ST/8) tiles (bf16) feed PV directly: outT[vd, q] = V_aug.T @ PT
    where V_aug = [V | ones] also yields the softmax denominator row
  - out-projection: out[s, e] = attnT.T @ Wo_g with attnT = normalized outT

Biases are all zeros and the mask is all ones per the problem spec, so both
are elided.  All matmuls are bf16 with fp32 PSUM accumulation.
"""

import numpy as np
import ml_dtypes
from contextlib import ExitStack

import concourse.bass as bass
import concourse.mybir as mybir
import concourse.tile as tile
from concourse import bacc
from concourse.bass_utils import run_bass_kernel_spmd
from concourse.masks import make_identity

D = 2048
HD = 64
R = 4
G = 8                   # kv heads == cores
CQ = R * HD             # 256: query-proj columns per core
NCH = D // 128          # 16 contraction chunks over d
BF16 = mybir.dt.bfloat16
F32 = mybir.dt.float32
EXPF = mybir.ActivationFunctionType.Exp

# set by test.py to collect a profile; harness path keeps defaults
TRACE = False
LAST_RESULTS = None


def build_nc(seq=2048):
    """Build the per-core Bass program (SPMD: same program, per-core data)."""
    NQB = seq // 512     # q blocks
    NKT = seq // 128     # k tiles
    NSB = seq // 512     # s blocks in projections

    nc = bacc.Bacc("TRN2", target_bir_lowering=False, debug=False)

    xT = nc.dram_tensor("xT", [D, seq], BF16, kind="ExternalInput")
    wq = nc.dram_tensor("wq", [D, CQ], BF16, kind="ExternalInput")
    wkv = nc.dram_tensor("wkv", [D, 128], BF16, kind="ExternalInput")
    wo = nc.dram_tensor("wo", [CQ, D], BF16, kind="ExternalInput")
    outp = nc.dram_tensor("outp", [seq, D], F32, kind="ExternalOutput")

    with ExitStack() as ctx:
        tc = ctx.enter_context(tile.TileContext(nc))
        singles = ctx.enter_context(tc.tile_pool(name="singles", bufs=1))
        # PSUM: scp = 3 x [128,1024] f32 (6 banks), acc = 2 x [128,512] (2 banks)
        scp = ctx.enter_context(
            tc.tile_pool(name="scp", bufs=3, space=bass.MemorySpace.PSUM)
        )
        acc = ctx.enter_context(
            tc.tile_pool(name="acc", bufs=2, space=bass.MemorySpace.PSUM)
        )
        ptp = ctx.enter_context(tc.tile_pool(name="ptp", bufs=NKT + 2))
        outsp = ctx.enter_context(tc.tile_pool(name="outsp", bufs=3))
        smp = ctx.enter_context(tc.tile_pool(name="smp", bufs=4))

        # persistent SBUF tensors
        xt = singles.tile([128, NCH, seq], BF16)          # x.T, d-chunked
        wq_sb = singles.tile([128, NCH, CQ], BF16)        # Wq_g
        wkv_sb = singles.tile([128, NCH, 128], BF16)      # [Wk_g | Wv_g]
        wo_sb = singles.tile([128, 2, D], BF16)           # Wo_g rows, c-chunked
        qt = singles.tile([128, 2, seq], BF16)            # QT: head-pair stacked
        kt_sb = singles.tile([128, seq], BF16)            # KT duplicated on parts
        vaug = singles.tile([128, NKT, 65], BF16)         # [V | ones] per k-chunk
        attnT = singles.tile([128, 2, seq], BF16)         # normalized attn^T
        ident = singles.tile([128, 128], BF16)

        make_identity(nc, ident[:])
        nc.vector.memset(vaug[:, :, 64:65], 1.0)

        # input loads (weights needed first, wo only for phase C)
        nc.sync.dma_start(
            out=wq_sb[:], in_=wq[:].rearrange("(c p) n -> p c n", p=128)
        )
        nc.sync.dma_start(
            out=wkv_sb[:], in_=wkv[:].rearrange("(c p) n -> p c n", p=128)
        )
        for ch in range(NCH):
            nc.sync.dma_start(out=xt[:, ch, :], in_=xT[ch * 128:(ch + 1) * 128, :])
        nc.sync.dma_start(
            out=wo_sb[:], in_=wo[:].rearrange("(c p) n -> p c n", p=128)
        )

        # ---- Phase A: projections ----
        # KV pass sink: rows 0-63 = KT, rows 64-127 = VT
        def kv_sink(sb, ssl, ps):
            nc.vector.tensor_copy(kt_sb[0:64, ssl], ps[0:64, :])
            vt_sb = outsp.tile([64, 512], BF16, tag="vt")
            nc.vector.tensor_copy(vt_sb[:], ps[64:128, :])
            for j in range(4):
                ktile = sb * 4 + j
                pst = acc.tile([128, 64], BF16, tag="ps")
                nc.tensor.transpose(
                    pst[:], vt_sb[:, j * 128:(j + 1) * 128], ident[0:64, 0:64]
                )
                nc.vector.tensor_copy(vaug[:, ktile, 0:64], pst[:])
            # duplicate KT onto partitions 64-127 for PE row-packing
            nc.gpsimd.dma_start(out=kt_sb[64:128, ssl], in_=kt_sb[0:64, ssl])

        def q_sink(hp):
            def sink(sb, ssl, ps):
                nc.vector.tensor_copy(qt[:, hp, ssl], ps[:, :])
            return sink

        # chains emitted chunk-outer in waves of 3 (parked in the otherwise
        # idle scp slots) so the PE rides just behind the streaming xT DMA
        # instead of stalling a full chain per chunk.
        chains = []
        for sb in range(NSB):
            chains.append((wkv_sb, slice(0, 128), sb, kv_sink))
        for sb in range(NSB):
            chains.append((wq_sb, slice(0, 128), sb, q_sink(0)))
        for sb in range(NSB):
            chains.append((wq_sb, slice(128, 256), sb, q_sink(1)))

        for w0 in range(0, len(chains), 3):
            wave = chains[w0:w0 + 3]
            pss = [scp.tile([128, 1024], F32, tag="sc", name=f"pswave{w0}_{i}") for i, _ in enumerate(wave)]
            for ch in range(NCH):
                for (w_sb, cols, sb, _sink), ps in zip(wave, pss):
                    ssl = slice(sb * 512, (sb + 1) * 512)
                    nc.tensor.matmul(
                        ps[:, 0:512],
                        w_sb[:, ch, cols],
                        xt[:, ch, ssl],
                        start=(ch == 0),
                        stop=(ch == NCH - 1),
                    )
            for (w_sb, cols, sb, sink), ps in zip(wave, pss):
                sink(sb, slice(sb * 512, (sb + 1) * 512), ps[:, 0:512])

        # ---- Phase B (attention) interleaved with Phase C (out-projection) ----
        # out-projection work for one 128-row s-tile, split into 4 eb-chains
        # that get woven into the ACT-limited PV stream of the next q-block
        obs = {}

        def c_chain(st, eb):
            esl = slice(eb * 512, (eb + 1) * 512)
            ssl = slice(st * 128, (st + 1) * 128)
            if eb == 0:
                obs[st] = outsp.tile([128, D], F32, tag="ob", name=f"ob{st}")
            ob = obs[st]
            ps = acc.tile([128, 512], F32, tag="ps")
            nc.tensor.matmul(
                ps[:], attnT[:, 0, ssl], wo_sb[:, 0, esl],
                start=True, stop=False,
            )
            nc.tensor.matmul(
                ps[:], attnT[:, 1, ssl], wo_sb[:, 1, esl],
                start=False, stop=True,
            )
            nc.vector.tensor_copy(ob[:, esl], ps[:])
            if eb == 3:
                nc.sync.dma_start(out=outp[ssl, :], in_=ob[:])
                del obs[st]

        # pending out-projection eb-chain state
        pending = []          # list of (st, eb)

        def queue_c(qb):
            for st in range(qb * 4, (qb + 1) * 4):
                for eb in range(4):
                    pending.append((st, eb))

        def drain_c(n):
            for _ in range(n):
                if pending:
                    c_chain(*pending.pop(0))

        for qb in range(NQB):
            qsl = slice(qb * 512, (qb + 1) * 512)
            for hp in range(2):
                # scores^T for heads (2hp, 2hp+1), row-packed on the PE:
                # head A weights on array rows 0-63, head B on rows 64-127
                pts = []
                for kt in range(NKT):
                    ksl = slice(kt * 128, (kt + 1) * 128)
                    ps = scp.tile([128, 1024], F32, tag="sc")
                    nc.tensor.matmul(
                        ps[:, 0:512], kt_sb[0:64, ksl], qt[0:64, hp, qsl],
                        start=True, stop=True,
                    )
                    nc.tensor.matmul(
                        ps[:, 512:1024], kt_sb[64:128, ksl], qt[64:128, hp, qsl],
                        start=True, stop=True,
                    )
                    pt = ptp.tile([128, 1024], BF16, tag="pt")
                    nc.scalar.activation(pt[:], ps[:], EXPF, scale=1.0 / 8.0)
                    pts.append(pt)

                # PV: outT[vd,q] (+ denominator row 64) for both heads.
                # PV matmul kt is gated on exp kt (ACT-limited), so weave in
                # the previous q-block's out-projection chains as PE filler.
                pv = scp.tile([128, 1024], F32, tag="sc")
                for kt in range(NKT):
                    nc.tensor.matmul(
                        pv[0:65, 0:512], vaug[:, kt, :], pts[kt][:, 0:512],
                        start=(kt == 0), stop=(kt == NKT - 1),
                    )
                    nc.tensor.matmul(
                        pv[0:65, 512:1024], vaug[:, kt, :], pts[kt][:, 512:1024],
                        start=(kt == 0), stop=(kt == NKT - 1),
                    )
                    if kt % 2 == 1:
                        drain_c(1)

                # normalize: attnT = outT * (1/denom), denom broadcast over
                # partitions on the (otherwise idle) GPSIMD engine
                for hb in range(2):
                    fsl = slice(hb * 512, (hb + 1) * 512)
                    rec = smp.tile([1, 512], F32, tag="rec")
                    nc.vector.reciprocal(rec[:], pv[64:65, fsl])
                    bc_sb = smp.tile([64, 512], F32, tag="bc")
                    nc.gpsimd.partition_broadcast(bc_sb[:], rec[:])
                    nc.vector.tensor_mul(
                        attnT[hb * 64:(hb + 1) * 64, hp, qsl],
                        pv[0:64, fsl],
                        bc_sb[:],
                    )

            # this q-block's attnT is final: queue its out-projection; the
            # chains drain inside the next q-block's PV (or right below for
            # the last one)
            drain_c(len(pending))
            queue_c(qb)
        drain_c(len(pending))

    nc.compile()
    return nc


_NC_CACHE = {}


def _get_nc(seq=2048):
    if seq not in _NC_CACHE:
        _NC_CACHE[seq] = build_nc(seq)
    return _NC_CACHE[seq]


def make_in_maps(x, Wq, Wk, Wv, Wo):
    """Host-side shard/marshal: bf16 casts, x transpose, per-core weight slices."""
    bf = ml_dtypes.bfloat16
    seq = x.shape[-2]
    x2 = np.asarray(x, np.float32).reshape(seq, D)
    xT = np.ascontiguousarray(x2.T).astype(bf)
    Wq = np.asarray(Wq, np.float32)
    Wk = np.asarray(Wk, np.float32)
    Wv = np.asarray(Wv, np.float32)
    Wo = np.asarray(Wo, np.float32)
    in_maps = []
    for g in range(G):
        wq_g = Wq[:, g * CQ:(g + 1) * CQ]
        wk_g = Wk[:, g * HD:(g + 1) * HD]
        wv_g = Wv[:, g * HD:(g + 1) * HD]
        in_maps.append({
            "xT": xT,
            "wq": np.ascontiguousarray(wq_g).astype(bf),
            "wkv": np.concatenate([wk_g, wv_g], axis=1).astype(bf),
            "wo": np.ascontiguousarray(Wo[g * CQ:(g + 1) * CQ, :]).astype(bf),
        })
    return in_maps


def kernel(x, mask, Wq, bq, Wk, bk, Wv, bv, Wo, bo):
    """Full-input entry point: shards across 8 NeuronCores, returns full output."""
    global LAST_RESULTS
    x = np.asarray(x)
    b, seq, d = x.shape
    assert d == D
    nc = _get_nc(seq)
    in_maps = make_in_maps(x, Wq, Wk, Wv, Wo)
    res = run_bass_kernel_spmd(nc, in_maps, core_ids=list(range(G)), trace=TRACE)
    LAST_RESULTS = res
    out = np.zeros((seq, D), np.float32)
    for r in res.results:
        out += r["outp"]
    return out.reshape(b, seq, D).astype(np.float32)

